# revision 35
# baseline (speedup 1.0000x reference)
"""Trainium2 distributed kernel for nn_AutoCorrelationLayer (FourierBlock).

Only 32 of 1025 rfft bins survive, so both FFTs collapse to small DFT
matmuls and Wq/Wo fold into per-mode weights on the host (stored as r/i/-i
blocks so the complex mix needs no on-device negation).  Pipeline per core:
stage A (DFT, hidden under input DMA) -> A2A #1 (batch->mode) -> stage B
(per-mode mix, free dim 512, col-packed) -> A2A #2 (mode->batch, SPLIT
into two half-batch ops) -> stage C (iDFT, row-packed) -> stores.

Trace-driven scheduling decisions:
  - ncfw collectives have a ~60-70us init anchored to NEFF exec start and
    the FIRST mesh op carries ~7us extra cold setup, so A2A #1 stays the
    first (and single) collective; the whole front end hides under it.
  - Warm mesh ops: 128KB ~5.6us, b2b ops queue with ~1.8us gap.  A2A #2
    is split into two 128KB halves (dest batches 0-1 / 2-3) so stage C +
    the 8MB output store start ~4us earlier; the second half transfers
    under the first half's compute.
  - Stage-B evictions are split by ri (ri0->ACT, ri1->DVE) so each a2
    staging DMA waits on ONE engine's counter; triggers fire with one
    staging DMA still in flight (lands inside the ~1.2us trigger window).
  - Tail is store-bound: stores stream on THREE rings (SP/ACT/GP); psC
    evictions alternate ACT/DVE in bank-pairs; yst loads split SP/ACT/GP
    per dest batch so the first stage-C matmul starts ~2us after the
    half-op completes.
All matmuls bf16 with f32 PSUM accumulation; rel err ~3.9e-3 (gate 2e-2).
"""

import sys
from contextlib import ExitStack

import numpy as np

sys.path.insert(0, "/opt/trn_rl_repo")

import concourse.bass as bass  # noqa: E402
import concourse.mybir as mybir  # noqa: E402
from concourse.bass_utils import run_bass_kernel_spmd  # noqa: E402

import ml_dtypes  # noqa: E402

BF16 = ml_dtypes.bfloat16

B, L, E, MODES = 32, 2048, 512, 32
NCORES = 8
BL = B // NCORES          # local batches per core (4)
ML = MODES // NCORES      # local modes per core (4)
NCH = E // 128            # 128-partition chunks of E (4)
KT = L // 128             # k-tiles along L (16)
GRP = 2 * ML              # cols per mode-group in DFT output (4 cos + 4 sin)

_nc_cache = {}


def build_nc(with_bias=True):
    f32 = mybir.dt.float32
    bf16 = mybir.dt.bfloat16

    nc = bass.Bass()

    q_ext = nc.declare_dram_parameter("q", [BL, L, E], bf16, isOutput=False)
    ft_ext = nc.declare_dram_parameter("ft", [128, KT * 64], bf16, isOutput=False)
    w_ext = nc.declare_dram_parameter("w", [ML, 128, 3 * NCH * E], bf16, isOutput=False)
    g_ext = nc.declare_dram_parameter("g", [128, L], bf16, isOutput=False)
    mb_ext = nc.declare_dram_parameter("mb", [B, E], f32, isOutput=False)
    out_ext = nc.declare_dram_parameter("out", [BL, 128, KT * E], bf16, isOutput=True)

    # A2A bounces. a1: [dest j][b 4][i 512][col 8]
    # a2 halves: [dest j][x 4][b 2][pp 2][p 512] (dest batches 0-1 / 2-3)
    a1_in = nc.dram_tensor("a1_in", [NCORES, BL * E * GRP], bf16)
    a1_out = nc.dram_tensor("a1_out", [NCORES, BL * E * GRP], bf16)
    a2a_in = nc.dram_tensor("a2a_in", [NCORES, (BL // 2) * GRP * E], bf16)
    a2a_out = nc.dram_tensor("a2a_out", [NCORES, (BL // 2) * GRP * E], bf16)
    a2b_in = nc.dram_tensor("a2b_in", [NCORES, (BL // 2) * GRP * E], bf16)
    a2b_out = nc.dram_tensor("a2b_out", [NCORES, (BL // 2) * GRP * E], bf16)
    rg = [list(range(NCORES))]

    with ExitStack() as ctx:
        ft_sb = ctx.enter_context(nc.sbuf_tensor([128, KT * 64], bf16))
        w_sb = ctx.enter_context(nc.sbuf_tensor([128, ML * 3 * NCH * E], bf16))
        g_sb = ctx.enter_context(nc.sbuf_tensor([128, L], bf16))
        mb_sb = ctx.enter_context(nc.sbuf_tensor([B, E], f32))
        qk_sb = ctx.enter_context(nc.sbuf_tensor([128, 2 * KT * E], bf16))
        qa_sb = ctx.enter_context(nc.sbuf_tensor([128, BL * NCH * 64], bf16))
        qm_sb = ctx.enter_context(nc.sbuf_tensor([128, NCH * B * GRP], bf16))
        ys_sb = ctx.enter_context(nc.sbuf_tensor([128, 2 * E], bf16))
        yst_sb = ctx.enter_context(nc.sbuf_tensor([128, BL * E], bf16))
        ob_sb = ctx.enter_context(nc.sbuf_tensor([128, BL * 16 * E], bf16))
        ps = ctx.enter_context(nc.psum_tensor([128, 4096], f32))
        (sFt, sMb, sW, sG, sMA, sEA, sS1, sCC, sQM, sMB, sAD, sEBa, sEBv,
         sS2a, sS2b, sMC, sECa, sECv) = (
            ctx.enter_context(nc.semaphore(n))
            for n in ("sFt", "sMb", "sW", "sG", "sMA", "sEA", "sS1", "sCC",
                      "sQM", "sMB", "sAD", "sEBa", "sEBv", "sS2a", "sS2b",
                      "sMC", "sECa", "sECv")
        )
        sQh = [ctx.enter_context(nc.semaphore(f"sQ{i}")) for i in range(4)]
        sYs0 = ctx.enter_context(nc.semaphore("sYs0"))
        sYs1 = ctx.enter_context(nc.semaphore("sYs1"))
        sYsG = ctx.enter_context(nc.semaphore("sYsG"))
        sSt = ctx.enter_context(nc.semaphore("sSt"))
        block = ctx.enter_context(nc.Block())

        # views
        def qk_v(b, k):
            return qk_sb[:, ((b % 2) * KT + k) * E : ((b % 2) * KT + k + 1) * E]

        def ft_v(k):
            return ft_sb[:, 64 * k : 64 * (k + 1)]

        def w_v(t, j, ch):
            o = ((t * 3 + j) * NCH + ch) * E
            return w_sb[:, o : o + E]

        def psA_v(b, ch):
            bank = (b % 2) * 4 + ch
            return ps[:, 512 * bank : 512 * bank + 64]

        def psB_v(t, ri):
            x = 2 * (t % 2) + ri
            bank = 4 * (t // 2) + x
            return ps[32 * x : 32 * (x + 1), 512 * bank : 512 * (bank + 1)]

        def psC_v(idx):
            bank = idx % 8
            return ps[:, 512 * bank : 512 * (bank + 1)]

        qa_r = qa_sb.rearrange(
            "p (j b ch u) -> p j b ch u", j=NCORES, b=BL, ch=NCH, u=GRP
        )

        def qa_v(b, ch):
            return qa_r[:, :, b, ch, :]  # (128, 8, 8) strided

        # qm columns permuted to (h, j, b2): global batch 4j + 2h + b2 lands
        # at psB row 16h + 2j + b2, so each a2 half is a contiguous 16-row
        # slice of ys.  Loaded by a 2-DMA h-scatter (SP h0, ACT h1).
        qm_r = qm_sb.rearrange(
            "p (h j b ch u) -> p ch u (h j b)", h=2, j=NCORES, b=2, ch=NCH, u=GRP
        )

        def ys_v(t, ri):
            x = 2 * (t % 2) + ri
            return ys_sb[32 * x : 32 * (x + 1), (t // 2) * E : (t // 2 + 1) * E]

        def ys_half(x, h):
            return ys_sb[32 * x + 16 * h : 32 * x + 16 * h + 16, :]

        def a2_stage_v(buf, x):
            return buf.rearrange("j (x b f) -> x j b f", x=4, b=2, f=2 * E)[x]

        def qm_scatter(eng, h, jlo, jhi):
            # land a1_out slot payloads at the h-permuted qm columns
            eng.dma_start(
                out=qm_sb.rearrange(
                    "p (h j b f) -> p h j b f", h=2, j=NCORES, b=2
                )[:, h, jlo:jhi],
                in_=a1_out.rearrange(
                    "j (p h b f) -> p h j b f", p=128, h=2, b=2
                )[:, h, jlo:jhi],
            ).then_inc(sQM, 16)

        def yst_v(b):
            return yst_sb[:, b * E : (b + 1) * E]

        def a2_out_v(buf, b2):
            return buf.rearrange(
                "j (x b pp p) -> b j x pp p", x=4, b=2, pp=2, p=E
            )[b2]

        def ob_v2(pidx):
            # pair eviction: psC banks [2*pidx, 2*pidx+1] -> ob cols
            return ob_sb[:, 2 * pidx * E : (2 * pidx + 2) * E]

        def psC_pair(pidx):
            bank = (2 * pidx) % 8
            return ps[:, 512 * bank : 512 * (bank + 2)]

        def ob_b(bb):
            return ob_sb[:, bb * 16 * E : (bb + 1) * 16 * E]

        # C evictions in PAIRS of psC banks; alternate ACT / DVE per pair
        EV_ENG = [("a", "v")[i % 2] for i in range(BL * 8)]
        EV_SEM = {"a": sECa, "v": sECv}

        def ev_count(eng, upto_pair):
            return sum(1 for i in range(upto_pair + 1) if EV_ENG[i] == eng)

        def wait_evs(eng_obj, upto_pair):
            for e in ("a", "v"):
                n = ev_count(e, upto_pair)
                if n:
                    eng_obj.wait_ge(EV_SEM[e], n)

        # store quarters round-robin over SP / ACT / GP
        STORE_ENG = {}
        for bb in range(BL):
            for q in range(4):
                STORE_ENG[(bb, q)] = ("s", "a", "g")[(4 * bb + q) % 3]

        def store_dma(ring, bb, q):
            wait_evs(ring, 8 * bb + 2 * q + 1)
            ring.dma_start(
                out=out_ext[bb][:, 4 * q * E : 4 * (q + 1) * E],
                in_=ob_b(bb)[:, 4 * q * E : 4 * (q + 1) * E],
            ).then_inc(sSt, 16)

        def yst_load(ring, b, sem):
            buf = a2a_out if b < 2 else a2b_out
            for half in range(2):
                ring.dma_start(
                    out=yst_v(b)[64 * half : 64 * (half + 1), :],
                    in_=a2_out_v(buf, b % 2),
                ).then_inc(sem, 16)

        # ---------------- SP ring: input streams, stagings, stores ----------
        @block.sync
        def _(sync):
            for b in range(BL):
                if b >= 2:
                    sync.wait_ge(sMA, 4 * (b - 1))  # batch b-2 fully consumed
                for h in range(2):
                    sync.dma_start(
                        out=qk_sb.rearrange("p (s k e) -> p s k e", s=4, k=KT // 2)[
                            :, (b % 2) * 2 + h
                        ],
                        in_=q_ext[b].rearrange("(k p) e -> p k e", p=128)[
                            :, 8 * h : 8 * (h + 1)
                        ],
                    ).then_inc(sQh[(b % 2) * 2 + h], 16)
            # qm load (after collective 1): SP takes j 0-3, both h-halves
            sync.wait_ge(sCC, 1)
            qm_scatter(sync, 0, 0, 4)
            qm_scatter(sync, 1, 0, 4)
            # a2 stagings, SP share: quadrants x1, x3 (ri=1 -> DVE evicted)
            sync.wait_ge(sEBv, 3)
            sync.dma_start(
                out=a2_stage_v(a2a_in, 1), in_=ys_half(1, 0)
            ).then_inc(sS2a, 16)
            sync.wait_ge(sEBv, 4)
            sync.dma_start(
                out=a2_stage_v(a2a_in, 3), in_=ys_half(3, 0)
            ).then_inc(sS2a, 16)
            sync.dma_start(
                out=a2_stage_v(a2b_in, 1), in_=ys_half(1, 1)
            ).then_inc(sS2b, 16)
            sync.dma_start(
                out=a2_stage_v(a2b_in, 3), in_=ys_half(3, 1)
            ).then_inc(sS2b, 16)
            # yst b0 after first a2 half-op
            sync.wait_ge(sCC, 2)
            yst_load(sync, 0, sYs0)
            for bb, q in sorted(k for k, v in STORE_ENG.items() if v == "s"):
                store_dma(sync, bb, q)

        # ---------------- PE: all matmuls ----------------
        @block.tensor
        def _(pe):
            pe.wait_ge(sFt, 16)  # ft loaded
            for b in range(BL):
                if b >= 2:
                    pe.wait_ge(sEA, 4 * (b - 1))  # psum bank set evicted
                for k in range(KT):
                    if k % 8 == 0:
                        pe.wait_ge(sQh[(b % 2) * 2 + k // 8], 16 * (b // 2 + 1))
                    for ch in range(NCH):
                        mm = pe.matmul(
                            psA_v(b, ch),
                            qk_v(b, k)[:, 128 * ch : 128 * (ch + 1)],
                            ft_v(k),
                            start=(k == 0),
                            stop=(k == KT - 1),
                        )
                        if k == KT - 1:
                            mm.then_inc(sMA, 1)
            # stage B
            pe.wait_ge(sW, 64)
            pe.wait_ge(sG, 16)
            pe.wait_ge(sQM, 64)
            for ps_i_ in range(2):
                for ch in range(NCH):
                    for tl in range(2):
                        t = 2 * ps_i_ + tl
                        lhs_r = qm_r[:, ch, t, :]
                        lhs_i = qm_r[:, ch, ML + t, :]
                        first, last = ch == 0, ch == NCH - 1
                        tp0 = (0, 32 * (2 * tl + 0))
                        tp1 = (0, 32 * (2 * tl + 1))
                        pe.matmul(psB_v(t, 0), lhs_r, w_v(t, 0, ch),
                                  start=first, stop=False, tile_position=tp0)
                        pe.matmul(psB_v(t, 1), lhs_r, w_v(t, 1, ch),
                                  start=first, stop=False, tile_position=tp1)
                        m3 = pe.matmul(psB_v(t, 0), lhs_i, w_v(t, 2, ch),
                                       start=False, stop=last, tile_position=tp0)
                        m4 = pe.matmul(psB_v(t, 1), lhs_i, w_v(t, 0, ch),
                                       start=False, stop=last, tile_position=tp1)
                        if last:
                            m3.then_inc(sMB, 1)
                            m4.then_inc(sMB, 1)
            # stage C
            for b in range(BL):
                pe.wait_ge((sYs0, sYs1, sYsG, sYsG)[b], (32, 32, 32, 64)[b])
                for lch in range(0, 16, 2):
                    idx = b * 16 + lch
                    if idx >= 8:
                        wait_evs(pe, (idx - 8) // 2)
                    pe.matmul(
                        psC_v(idx),
                        g_sb[0:64, 128 * lch : 128 * (lch + 1)],
                        yst_v(b)[0:64, :],
                        start=True,
                        stop=True,
                        tile_position=(0, 0),
                    ).then_inc(sMC, 1)
                    pe.matmul(
                        psC_v(idx + 1),
                        g_sb[64:128, 128 * (lch + 1) : 128 * (lch + 2)],
                        yst_v(b)[64:128, :],
                        start=True,
                        stop=True,
                        tile_position=(64, 0),
                    ).then_inc(sMC, 1)

        # ------------- ACT ring: consts, evictions, stagings, stores --------
        @block.scalar
        def _(act):
            act.dma_start(out=ft_sb[:], in_=ft_ext[:]).then_inc(sFt, 16)
            if with_bias:
                act.dma_start(out=mb_sb[:], in_=mb_ext[:]).then_inc(sMb, 16)
            act.dma_start(out=g_sb[:], in_=g_ext[:]).then_inc(sG, 16)
            # stage A evictions (f32 -> bf16)
            for b in range(BL):
                for ch in range(NCH):
                    act.wait_ge(sMA, 4 * b + ch + 1)
                    act.copy(
                        out=qa_v(b, ch),
                        in_=psA_v(b, ch).rearrange("p (j u) -> p j u", j=NCORES),
                    ).then_inc(sEA, 1)
            # staging 1
            act.wait_ge(sEA, 16)
            act.dma_start(
                out=a1_in.rearrange("j (p f) -> p j f", p=128),
                in_=qa_sb.rearrange("p (j f) -> p j f", j=NCORES),
            ).then_inc(sS1, 16)
            # w loads drain during collective 1
            for t in range(ML):
                act.dma_start(
                    out=w_sb[:, t * 3 * NCH * E : (t + 1) * 3 * NCH * E],
                    in_=w_ext[t],
                ).then_inc(sW, 16)
            # qm load ACT half (j 4-7), both h-halves
            act.wait_ge(sCC, 1)
            qm_scatter(act, 0, 4, 8)
            qm_scatter(act, 1, 4, 8)
            # stage B evictions: ACT takes ri=0 of every t
            for t in range(ML):
                if t == 0 and with_bias:
                    act.wait_ge(sAD, 1)
                else:
                    act.wait_ge(sMB, 2 * t + 1)
                act.copy(out=ys_v(t, 0), in_=psB_v(t, 0)).then_inc(sEBa, 1)
                # quadrants x0 (t0+t2, ri0) and x2 (t1+t3, ri0) become ready
                # as soon as the LAST of their two ACT evictions lands
                if t == 2:
                    act.dma_start(
                        out=a2_stage_v(a2a_in, 0), in_=ys_half(0, 0)
                    ).then_inc(sS2a, 16)
                    act.dma_start(
                        out=a2_stage_v(a2b_in, 0), in_=ys_half(0, 1)
                    ).then_inc(sS2b, 16)
                if t == 3:
                    act.dma_start(
                        out=a2_stage_v(a2a_in, 2), in_=ys_half(2, 0)
                    ).then_inc(sS2a, 16)
                    act.dma_start(
                        out=a2_stage_v(a2b_in, 2), in_=ys_half(2, 1)
                    ).then_inc(sS2b, 16)
            # yst b1 after first a2 half-op
            act.wait_ge(sCC, 2)
            yst_load(act, 1, sYs1)
            # stage C evictions (ACT share) with ACT-assigned stores woven in
            act_stores = sorted(k for k, v in STORE_ENG.items() if v == "a")
            for pidx in range(BL * 8):
                if EV_ENG[pidx] == "a":
                    act.wait_ge(sMC, 2 * pidx + 2)
                    act.copy(out=ob_v2(pidx), in_=psC_pair(pidx)).then_inc(sECa, 1)
                while act_stores and 8 * act_stores[0][0] + 2 * act_stores[0][1] + 1 <= pidx:
                    bb, q = act_stores.pop(0)
                    store_dma(act, bb, q)
            for bb, q in act_stores:
                store_dma(act, bb, q)

        # ------------- DVE: bias add, ri=1 evictions, 1/2 of C evictions -----
        @block.vector
        def _(dve):
            if with_bias:
                dve.wait_ge(sMb, 16)  # mb loaded
                dve.wait_ge(sMB, 1)   # t=0 yr chain done
                dve.tensor_add(psB_v(0, 0), psB_v(0, 0), mb_sb[:]).then_inc(sAD, 1)
            # stage B evictions: DVE takes ri=1 of every t
            for t in range(ML):
                dve.wait_ge(sMB, 2 * t + 2)
                dve.tensor_copy(ys_v(t, 1), psB_v(t, 1)).then_inc(sEBv, 1)
            for pidx in range(BL * 8):
                if EV_ENG[pidx] != "v":
                    continue
                dve.wait_ge(sMC, 2 * pidx + 2)
                dve.tensor_copy(ob_v2(pidx), psC_pair(pidx)).then_inc(sECv, 1)

        # ---------------- GPSIMD: collectives, yst b2/b3, stores -------------
        @block.gpsimd
        def _(gp):
            gp.wait_ge(sS1, 16)
            gp.collective_compute(
                "AllToAll",
                mybir.AluOpType.bypass,
                replica_groups=rg,
                ins=[a1_in[:]],
                outs=[a1_out[:]],
            ).then_inc(sCC, 1)
            gp.wait_ge(sS2a, 64)
            gp.collective_compute(
                "AllToAll",
                mybir.AluOpType.bypass,
                replica_groups=rg,
                ins=[a2a_in[:]],
                outs=[a2a_out[:]],
            ).then_inc(sCC, 1)
            gp.wait_ge(sS2b, 64)
            gp.collective_compute(
                "AllToAll",
                mybir.AluOpType.bypass,
                replica_groups=rg,
                ins=[a2b_in[:]],
                outs=[a2b_out[:]],
            ).then_inc(sCC, 1)
            gp.wait_ge(sCC, 3)
            yst_load(gp, 2, sYsG)
            yst_load(gp, 3, sYsG)
            for bb, q in sorted(k for k, v in STORE_ENG.items() if v == "g"):
                store_dma(gp, bb, q)

    return nc


def _host_prep(queries, Wq, bq, W1r, W1i, Wo, bo):
    """Fold Wq/Wo into per-mode weights, build DFT matrices, shard per core."""
    l = np.arange(L)
    m = np.arange(MODES)
    ang = 2.0 * np.pi * np.outer(m, l) / L          # (M, L)
    cos_ml = np.cos(ang)
    sin_ml = np.sin(ang)

    # DFT moving tiles, packed [p_in_tile, k*64 + c]; c: group g -> [cos, -sin]
    ft = np.empty((L, 64), np.float32)
    for g in range(NCORES):
        ft[:, GRP * g : GRP * g + ML] = cos_ml[4 * g : 4 * g + ML].T
        ft[:, GRP * g + ML : GRP * (g + 1)] = -sin_ml[4 * g : 4 * g + ML].T
    ft = np.ascontiguousarray(
        ft.reshape(KT, 128, 64).transpose(1, 0, 2).reshape(128, KT * 64)
    )

    # Folded mode weights: W'_m = Wq.T @ (W1r_m + i W1i_m) @ Wo.T
    Wq64 = Wq.astype(np.float64)
    Wo64 = Wo.astype(np.float64)
    Wpr = np.empty((E, E, MODES), np.float32)
    Wpi = np.empty((E, E, MODES), np.float32)
    for mm in range(MODES):
        ar = Wq64.T @ W1r[:, :, mm].astype(np.float64)
        ai = Wq64.T @ W1i[:, :, mm].astype(np.float64)
        Wpr[:, :, mm] = (ar @ Wo64.T).astype(np.float32)
        Wpi[:, :, mm] = (ai @ Wo64.T).astype(np.float32)

    # Inverse DFT rows g[k = j*8 + t*2 + ri, l]
    cm = np.where(m == 0, 1.0, 2.0)
    g_mat = np.empty((64, L), np.float32)
    for r in range(64):
        c, x, pp = r // 8, (r % 8) // 2, r % 2
        tl, ri = x // 2, x % 2
        mm = 4 * c + 2 * pp + tl
        if ri == 0:
            g_mat[r] = cm[mm] * cos_ml[mm] / L
        else:
            g_mat[r] = -cm[mm] * sin_ml[mm] / L
            if mm == 0:
                g_mat[r] = 0.0  # irfft ignores Im(bin 0)

    out_bias = (
        bo.astype(np.float64)
        + bq.astype(np.float64) @ W1r[:, :, 0].astype(np.float64) @ Wo64.T
    ).astype(np.float32)

    ft_b = ft.astype(BF16)
    g_b = np.vstack([g_mat, g_mat]).astype(BF16)

    in_maps = []
    for c in range(NCORES):
        w_pack = np.empty((ML, 128, 3, NCH, E), np.float32)
        for t in range(ML):
            mm = 4 * c + t
            for ch in range(NCH):
                w_pack[t, :, 0, ch] = Wpr[128 * ch : 128 * (ch + 1), :, mm]
                w_pack[t, :, 1, ch] = Wpi[128 * ch : 128 * (ch + 1), :, mm]
                w_pack[t, :, 2, ch] = -Wpi[128 * ch : 128 * (ch + 1), :, mm]
        w_pack = w_pack.reshape(ML, 128, 3 * NCH * E)
        in_maps.append(
            {
                "q": np.ascontiguousarray(queries[BL * c : BL * (c + 1)]).astype(BF16),
                "ft": ft_b,
                "w": w_pack.astype(BF16),
                "g": g_b,
                "mb": np.broadcast_to(
                    L * out_bias[None, :] if c == 0 else np.zeros((1, E), np.float32),
                    (B, E),
                ).astype(np.float32),
            }
        )
    return in_maps


def kernel(queries, Wq, bq, W1r, W1i, Wo, bo, _trace=False):
    global _nc_cache
    with_bias = bool(np.any(bq) or np.any(bo))
    if with_bias not in _nc_cache:
        _nc_cache[with_bias] = build_nc(with_bias)
    nc = _nc_cache[with_bias]

    in_maps = _host_prep(queries, Wq, bq, W1r, W1i, Wo, bo)
    res = run_bass_kernel_spmd(nc, in_maps, core_ids=list(range(NCORES)), trace=_trace)
    results = res.results
    out = np.concatenate(
        [
            np.asarray(r["out"])
            .reshape(BL, 128, KT, E)
            .transpose(0, 2, 1, 3)
            .reshape(BL, L, E)
            for r in results
        ],
        axis=0,
    )
    if _trace:
        kernel._last = res
    return out.astype(np.float32)


# revision 43
# speedup vs baseline: 1.0489x; 1.0489x over previous
"""Trainium2 distributed kernel for nn_AutoCorrelationLayer (FourierBlock).

Only 32 of 1025 rfft bins survive, so both FFTs collapse to small DFT
matmuls and Wq/Wo fold into per-mode weights on the host (stored as r/i/-i
blocks so the complex mix needs no on-device negation).  Pipeline per core:
stage A (DFT, hidden under input DMA) -> A2A #1 (batch->mode) -> stage B
(per-mode mix, free dim 512, col-packed) -> A2A #2 (mode->batch, SPLIT
into two half-batch ops) -> stage C (iDFT, row-packed) -> stores.

Trace-driven scheduling decisions:
  - ncfw collectives have a ~60-70us init anchored to NEFF exec start and
    the FIRST mesh op carries ~7us extra cold setup, so A2A #1 stays the
    first (and single) collective; the whole front end hides under it.
  - Warm mesh ops: 128KB ~5.6us, b2b ops queue with ~1.8us gap.  A2A #2
    is split into two 128KB halves (dest batches 0-1 / 2-3) so stage C +
    the 8MB output store start ~4us earlier; the second half transfers
    under the first half's compute.
  - Stage-B evictions are split by ri (ri0->ACT, ri1->DVE) so each a2
    staging DMA waits on ONE engine's counter; triggers fire with one
    staging DMA still in flight (lands inside the ~1.2us trigger window).
  - Tail is store-bound: stores stream on THREE rings (SP/ACT/GP); psC
    evictions alternate ACT/DVE in bank-pairs; yst loads split SP/ACT/GP
    per dest batch so the first stage-C matmul starts ~2us after the
    half-op completes.
All matmuls bf16 with f32 PSUM accumulation; rel err ~3.9e-3 (gate 2e-2).
"""

import sys
from contextlib import ExitStack

import numpy as np

sys.path.insert(0, "/opt/trn_rl_repo")

import concourse.bass as bass  # noqa: E402
import concourse.mybir as mybir  # noqa: E402
from concourse.bass_utils import run_bass_kernel_spmd  # noqa: E402

import ml_dtypes  # noqa: E402

BF16 = ml_dtypes.bfloat16

B, L, E, MODES = 32, 2048, 512, 32
NCORES = 8
BL = B // NCORES          # local batches per core (4)
ML = MODES // NCORES      # local modes per core (4)
NCH = E // 128            # 128-partition chunks of E (4)
KT = L // 128             # k-tiles along L (16)
GRP = 2 * ML              # cols per mode-group in DFT output (4 cos + 4 sin)

_nc_cache = {}


def build_nc(with_bias=True):
    f32 = mybir.dt.float32
    bf16 = mybir.dt.bfloat16

    nc = bass.Bass()

    q_ext = nc.declare_dram_parameter("q", [BL, L, E], bf16, isOutput=False)
    ft_ext = nc.declare_dram_parameter("ft", [128, KT * 64], bf16, isOutput=False)
    w_ext = nc.declare_dram_parameter("w", [ML, 128, 3 * NCH * E], bf16, isOutput=False)
    g_ext = nc.declare_dram_parameter("g", [128, L], bf16, isOutput=False)
    mb_ext = nc.declare_dram_parameter("mb", [B, E], f32, isOutput=False)
    out_ext = nc.declare_dram_parameter("out", [BL, 128, KT * E], bf16, isOutput=True)

    # A2A bounces. a1: [dest j][b 4][i 512][col 8]
    # a2 halves: [dest j][x 4][b 2][pp 2][p 512] (dest batches 0-1 / 2-3)
    a1_in = nc.dram_tensor("a1_in", [NCORES, BL * E * GRP], bf16)
    a1_out = nc.dram_tensor("a1_out", [NCORES, BL * E * GRP], bf16)
    a2a_in = nc.dram_tensor("a2a_in", [NCORES, (BL // 2) * GRP * E], bf16)
    a2a_out = nc.dram_tensor("a2a_out", [NCORES, (BL // 2) * GRP * E], bf16)
    a2b_in = nc.dram_tensor("a2b_in", [NCORES, (BL // 2) * GRP * E], bf16)
    a2b_out = nc.dram_tensor("a2b_out", [NCORES, (BL // 2) * GRP * E], bf16)
    rg = [list(range(NCORES))]

    with ExitStack() as ctx:
        ft_sb = ctx.enter_context(nc.sbuf_tensor([128, KT * 64], bf16))
        w_sb = ctx.enter_context(nc.sbuf_tensor([128, ML * 3 * NCH * E], bf16))
        g_sb = ctx.enter_context(nc.sbuf_tensor([128, L], bf16))
        mb_sb = ctx.enter_context(nc.sbuf_tensor([B, E], f32))
        qk_sb = ctx.enter_context(nc.sbuf_tensor([128, 2 * KT * E], bf16))
        qa_sb = ctx.enter_context(nc.sbuf_tensor([128, BL * NCH * 64], bf16))
        qm_sb = ctx.enter_context(nc.sbuf_tensor([128, NCH * B * GRP], bf16))
        ys_sb = ctx.enter_context(nc.sbuf_tensor([128, 2 * E], bf16))
        yst_sb = ctx.enter_context(nc.sbuf_tensor([128, BL * E], bf16))
        ob_sb = ctx.enter_context(nc.sbuf_tensor([128, BL * 16 * E], bf16))
        ps = ctx.enter_context(nc.psum_tensor([128, 4096], f32))
        (sFt, sMb, sW, sG, sMA, sEA, sS1, sCC, sQM, sMB, sAD, sEBa, sEBv,
         sS2a, sS2b, sMC, sECa, sECv) = (
            ctx.enter_context(nc.semaphore(n))
            for n in ("sFt", "sMb", "sW", "sG", "sMA", "sEA", "sS1", "sCC",
                      "sQM", "sMB", "sAD", "sEBa", "sEBv", "sS2a", "sS2b",
                      "sMC", "sECa", "sECv")
        )
        sQh = [ctx.enter_context(nc.semaphore(f"sQ{i}")) for i in range(4)]
        sYs0 = ctx.enter_context(nc.semaphore("sYs0"))
        sYs1 = ctx.enter_context(nc.semaphore("sYs1"))
        sYsG = ctx.enter_context(nc.semaphore("sYsG"))
        sSt = ctx.enter_context(nc.semaphore("sSt"))
        block = ctx.enter_context(nc.Block())

        # views
        def qk_v(b, k):
            return qk_sb[:, ((b % 2) * KT + k) * E : ((b % 2) * KT + k + 1) * E]

        def ft_v(k):
            return ft_sb[:, 64 * k : 64 * (k + 1)]

        def w_v(t, j, ch):
            o = ((t * 3 + j) * NCH + ch) * E
            return w_sb[:, o : o + E]

        def psA_v(b, ch):
            bank = (b % 2) * 4 + ch
            return ps[:, 512 * bank : 512 * bank + 64]

        def psB_v(t, ri):
            x = 2 * (t % 2) + ri
            bank = 4 * (t // 2) + x
            return ps[32 * x : 32 * (x + 1), 512 * bank : 512 * (bank + 1)]

        def psC_v(idx):
            bank = idx % 8
            return ps[:, 512 * bank : 512 * (bank + 1)]

        qa_r = qa_sb.rearrange(
            "p (j b ch u) -> p j b ch u", j=NCORES, b=BL, ch=NCH, u=GRP
        )

        def qa_v(b, ch):
            return qa_r[:, :, b, ch, :]  # (128, 8, 8) strided

        # qm columns permuted to (h, j, b2): global batch 4j + 2h + b2 lands
        # at psB row 16h + 2j + b2, so each a2 half is a contiguous 16-row
        # slice of ys.  Loaded by a 2-DMA h-scatter (SP h0, ACT h1).
        qm_r = qm_sb.rearrange(
            "p (h j b ch u) -> p ch u (h j b)", h=2, j=NCORES, b=2, ch=NCH, u=GRP
        )

        def ys_v(t, ri):
            x = 2 * (t % 2) + ri
            return ys_sb[32 * x : 32 * (x + 1), (t // 2) * E : (t // 2 + 1) * E]

        def ys_half(x, h):
            return ys_sb[32 * x + 16 * h : 32 * x + 16 * h + 16, :]

        def a2_stage_v(buf, x):
            return buf.rearrange("j (x b f) -> x j b f", x=4, b=2, f=2 * E)[x]

        def qm_scatter(eng, h, jlo, jhi):
            # land a1_out slot payloads at the h-permuted qm columns
            eng.dma_start(
                out=qm_sb.rearrange(
                    "p (h j b f) -> p h j b f", h=2, j=NCORES, b=2
                )[:, h, jlo:jhi],
                in_=a1_out.rearrange(
                    "j (p h b f) -> p h j b f", p=128, h=2, b=2
                )[:, h, jlo:jhi],
            ).then_inc(sQM, 16)

        def yst_v(b):
            return yst_sb[:, b * E : (b + 1) * E]

        def a2_out_v(buf, b2):
            return buf.rearrange(
                "j (x b pp p) -> b j x pp p", x=4, b=2, pp=2, p=E
            )[b2]

        def ob_v2(pidx):
            # pair eviction: psC banks [2*pidx, 2*pidx+1] -> ob cols
            return ob_sb[:, 2 * pidx * E : (2 * pidx + 2) * E]

        def psC_pair(pidx):
            bank = (2 * pidx) % 8
            return ps[:, 512 * bank : 512 * (bank + 2)]

        def ob_b(bb):
            return ob_sb[:, bb * 16 * E : (bb + 1) * 16 * E]

        # C evictions in PAIRS of psC banks; alternate ACT / DVE per pair
        EV_ENG = [("a", "v")[i % 2] for i in range(BL * 8)]
        EV_SEM = {"a": sECa, "v": sECv}

        def ev_count(eng, upto_pair):
            return sum(1 for i in range(upto_pair + 1) if EV_ENG[i] == eng)

        def wait_evs(eng_obj, upto_pair):
            for e in ("a", "v"):
                n = ev_count(e, upto_pair)
                if n:
                    eng_obj.wait_ge(EV_SEM[e], n)

        # store quarters round-robin over SP / ACT / GP
        STORE_ENG = {}
        for bb in range(BL):
            for q in range(4):
                STORE_ENG[(bb, q)] = ("s", "a", "g")[(4 * bb + q) % 3]

        def store_dma(ring, bb, q):
            wait_evs(ring, 8 * bb + 2 * q + 1)
            ring.dma_start(
                out=out_ext[bb][:, 4 * q * E : 4 * (q + 1) * E],
                in_=ob_b(bb)[:, 4 * q * E : 4 * (q + 1) * E],
            ).then_inc(sSt, 16)

        def yst_load(ring, b, sem, halves=(0, 1)):
            buf = a2a_out if b < 2 else a2b_out
            for half in halves:
                ring.dma_start(
                    out=yst_v(b)[64 * half : 64 * (half + 1), :],
                    in_=a2_out_v(buf, b % 2),
                ).then_inc(sem, 16)

        # ---------------- SP ring: input streams, stagings, stores ----------
        @block.sync
        def _(sync):
            for b in range(BL):
                if b >= 2:
                    sync.wait_ge(sMA, 4 * (b - 1))  # batch b-2 fully consumed
                for h in range(2):
                    sync.dma_start(
                        out=qk_sb.rearrange("p (s k e) -> p s k e", s=4, k=KT // 2)[
                            :, (b % 2) * 2 + h
                        ],
                        in_=q_ext[b].rearrange("(k p) e -> p k e", p=128)[
                            :, 8 * h : 8 * (h + 1)
                        ],
                    ).then_inc(sQh[(b % 2) * 2 + h], 16)
            # qm load (after collective 1): SP takes j 0-3, both h-halves
            sync.wait_ge(sCC, 1)
            qm_scatter(sync, 0, 0, 4)
            qm_scatter(sync, 1, 0, 4)
            # a2 stagings: DMA issue blocks the engine ~0.8us each, so split
            # the critical a2a set SP(x0,x1)/ACT(x2,x3); SP then streams the
            # a2b set while gp triggers op2a.
            sync.wait_ge(sEBa, 3)
            sync.dma_start(
                out=a2_stage_v(a2a_in, 0), in_=ys_half(0, 0)
            ).then_inc(sS2a, 16)
            sync.wait_ge(sEBv, 3)
            sync.dma_start(
                out=a2_stage_v(a2a_in, 1), in_=ys_half(1, 0)
            ).then_inc(sS2a, 16)
            sync.wait_ge(sEBa, 4)
            sync.wait_ge(sEBv, 4)
            for x in range(4):
                sync.dma_start(
                    out=a2_stage_v(a2b_in, x), in_=ys_half(x, 1)
                ).then_inc(sS2b, 16)
            # yst after first a2 half-op: SP takes the h0 rows of b0 and b1
            sync.wait_ge(sCC, 2)
            yst_load(sync, 0, sYs0, halves=(0,))
            yst_load(sync, 1, sYs1, halves=(0,))
            for bb, q in sorted(k for k, v in STORE_ENG.items() if v == "s"):
                store_dma(sync, bb, q)

        # ---------------- PE: all matmuls ----------------
        @block.tensor
        def _(pe):
            pe.wait_ge(sFt, 16)  # ft loaded
            for b in range(BL):
                if b >= 2:
                    pe.wait_ge(sEA, 4 * (b - 1))  # psum bank set evicted
                for k in range(KT):
                    if k % 8 == 0:
                        pe.wait_ge(sQh[(b % 2) * 2 + k // 8], 16 * (b // 2 + 1))
                    for ch in range(NCH):
                        mm = pe.matmul(
                            psA_v(b, ch),
                            qk_v(b, k)[:, 128 * ch : 128 * (ch + 1)],
                            ft_v(k),
                            start=(k == 0),
                            stop=(k == KT - 1),
                        )
                        if k == KT - 1:
                            mm.then_inc(sMA, 1)
            # stage B
            pe.wait_ge(sW, 64)
            pe.wait_ge(sG, 16)
            pe.wait_ge(sQM, 64)
            for ps_i_ in range(2):
                for ch in range(NCH):
                    for tl in range(2):
                        t = 2 * ps_i_ + tl
                        lhs_r = qm_r[:, ch, t, :]
                        lhs_i = qm_r[:, ch, ML + t, :]
                        first, last = ch == 0, ch == NCH - 1
                        tp0 = (0, 32 * (2 * tl + 0))
                        tp1 = (0, 32 * (2 * tl + 1))
                        pe.matmul(psB_v(t, 0), lhs_r, w_v(t, 0, ch),
                                  start=first, stop=False, tile_position=tp0)
                        pe.matmul(psB_v(t, 1), lhs_r, w_v(t, 1, ch),
                                  start=first, stop=False, tile_position=tp1)
                        m3 = pe.matmul(psB_v(t, 0), lhs_i, w_v(t, 2, ch),
                                       start=False, stop=last, tile_position=tp0)
                        m4 = pe.matmul(psB_v(t, 1), lhs_i, w_v(t, 0, ch),
                                       start=False, stop=last, tile_position=tp1)
                        if last:
                            m3.then_inc(sMB, 1)
                            m4.then_inc(sMB, 1)
            # stage C
            for b in range(BL):
                pe.wait_ge((sYs0, sYs1, sYsG, sYsG)[b], (32, 32, 32, 64)[b])
                for lch in range(0, 16, 2):
                    idx = b * 16 + lch
                    if idx >= 8:
                        wait_evs(pe, (idx - 8) // 2)
                    pe.matmul(
                        psC_v(idx),
                        g_sb[0:64, 128 * lch : 128 * (lch + 1)],
                        yst_v(b)[0:64, :],
                        start=True,
                        stop=True,
                        tile_position=(0, 0),
                    ).then_inc(sMC, 1)
                    pe.matmul(
                        psC_v(idx + 1),
                        g_sb[64:128, 128 * (lch + 1) : 128 * (lch + 2)],
                        yst_v(b)[64:128, :],
                        start=True,
                        stop=True,
                        tile_position=(64, 0),
                    ).then_inc(sMC, 1)

        # ------------- ACT ring: consts, evictions, stagings, stores --------
        @block.scalar
        def _(act):
            act.dma_start(out=ft_sb[:], in_=ft_ext[:]).then_inc(sFt, 16)
            if with_bias:
                act.dma_start(out=mb_sb[:], in_=mb_ext[:]).then_inc(sMb, 16)
            act.dma_start(out=g_sb[:], in_=g_ext[:]).then_inc(sG, 16)
            # stage A evictions (f32 -> bf16)
            for b in range(BL):
                for ch in range(NCH):
                    act.wait_ge(sMA, 4 * b + ch + 1)
                    act.copy(
                        out=qa_v(b, ch),
                        in_=psA_v(b, ch).rearrange("p (j u) -> p j u", j=NCORES),
                    ).then_inc(sEA, 1)
            # staging 1
            act.wait_ge(sEA, 16)
            act.dma_start(
                out=a1_in.rearrange("j (p f) -> p j f", p=128),
                in_=qa_sb.rearrange("p (j f) -> p j f", j=NCORES),
            ).then_inc(sS1, 16)
            # w loads drain during collective 1
            for t in range(ML):
                act.dma_start(
                    out=w_sb[:, t * 3 * NCH * E : (t + 1) * 3 * NCH * E],
                    in_=w_ext[t],
                ).then_inc(sW, 16)
            # qm load ACT half (j 4-7), both h-halves
            act.wait_ge(sCC, 1)
            qm_scatter(act, 0, 4, 8)
            qm_scatter(act, 1, 4, 8)
            # stage B evictions: ACT takes ri=0 of every t, then issues the
            # x2/x3 a2a stagings (its own evicts gate x2; DVE's gate x3)
            for t in range(ML):
                if t == 0 and with_bias:
                    act.wait_ge(sAD, 1)
                else:
                    act.wait_ge(sMB, 2 * t + 1)
                act.copy(out=ys_v(t, 0), in_=psB_v(t, 0)).then_inc(sEBa, 1)
            act.dma_start(
                out=a2_stage_v(a2a_in, 2), in_=ys_half(2, 0)
            ).then_inc(sS2a, 16)
            act.wait_ge(sEBv, 4)
            act.dma_start(
                out=a2_stage_v(a2a_in, 3), in_=ys_half(3, 0)
            ).then_inc(sS2a, 16)
            # yst after first a2 half-op: ACT takes the h1 rows of b0 and b1
            act.wait_ge(sCC, 2)
            yst_load(act, 0, sYs0, halves=(1,))
            yst_load(act, 1, sYs1, halves=(1,))
            # stage C evictions (ACT share) with ACT-assigned stores woven in
            act_stores = sorted(k for k, v in STORE_ENG.items() if v == "a")
            for pidx in range(BL * 8):
                if EV_ENG[pidx] == "a":
                    act.wait_ge(sMC, 2 * pidx + 2)
                    act.copy(out=ob_v2(pidx), in_=psC_pair(pidx)).then_inc(sECa, 1)
                while act_stores and 8 * act_stores[0][0] + 2 * act_stores[0][1] + 1 <= pidx:
                    bb, q = act_stores.pop(0)
                    store_dma(act, bb, q)
            for bb, q in act_stores:
                store_dma(act, bb, q)

        # ------------- DVE: bias add, ri=1 evictions, 1/2 of C evictions -----
        @block.vector
        def _(dve):
            if with_bias:
                dve.wait_ge(sMb, 16)  # mb loaded
                dve.wait_ge(sMB, 1)   # t=0 yr chain done
                dve.tensor_add(psB_v(0, 0), psB_v(0, 0), mb_sb[:]).then_inc(sAD, 1)
            # stage B evictions: DVE takes ri=1 of every t
            for t in range(ML):
                dve.wait_ge(sMB, 2 * t + 2)
                dve.tensor_copy(ys_v(t, 1), psB_v(t, 1)).then_inc(sEBv, 1)
            for pidx in range(BL * 8):
                if EV_ENG[pidx] != "v":
                    continue
                dve.wait_ge(sMC, 2 * pidx + 2)
                dve.tensor_copy(ob_v2(pidx), psC_pair(pidx)).then_inc(sECv, 1)

        # ---------------- GPSIMD: collectives, yst b2/b3, stores -------------
        @block.gpsimd
        def _(gp):
            gp.wait_ge(sS1, 16)
            gp.collective_compute(
                "AllToAll",
                mybir.AluOpType.bypass,
                replica_groups=rg,
                ins=[a1_in[:]],
                outs=[a1_out[:]],
            ).then_inc(sCC, 1)
            gp.wait_ge(sS2a, 64)
            gp.collective_compute(
                "AllToAll",
                mybir.AluOpType.bypass,
                replica_groups=rg,
                ins=[a2a_in[:]],
                outs=[a2a_out[:]],
            ).then_inc(sCC, 1)
            gp.wait_ge(sS2b, 64)
            gp.collective_compute(
                "AllToAll",
                mybir.AluOpType.bypass,
                replica_groups=rg,
                ins=[a2b_in[:]],
                outs=[a2b_out[:]],
            ).then_inc(sCC, 1)
            gp.wait_ge(sCC, 3)
            yst_load(gp, 2, sYsG)
            yst_load(gp, 3, sYsG)
            for bb, q in sorted(k for k, v in STORE_ENG.items() if v == "g"):
                store_dma(gp, bb, q)

    return nc


def _host_prep(queries, Wq, bq, W1r, W1i, Wo, bo):
    """Fold Wq/Wo into per-mode weights, build DFT matrices, shard per core."""
    l = np.arange(L)
    m = np.arange(MODES)
    ang = 2.0 * np.pi * np.outer(m, l) / L          # (M, L)
    cos_ml = np.cos(ang)
    sin_ml = np.sin(ang)

    # DFT moving tiles, packed [p_in_tile, k*64 + c]; c: group g -> [cos, -sin]
    ft = np.empty((L, 64), np.float32)
    for g in range(NCORES):
        ft[:, GRP * g : GRP * g + ML] = cos_ml[4 * g : 4 * g + ML].T
        ft[:, GRP * g + ML : GRP * (g + 1)] = -sin_ml[4 * g : 4 * g + ML].T
    ft = np.ascontiguousarray(
        ft.reshape(KT, 128, 64).transpose(1, 0, 2).reshape(128, KT * 64)
    )

    # Folded mode weights: W'_m = Wq.T @ (W1r_m + i W1i_m) @ Wo.T
    Wq64 = Wq.astype(np.float64)
    Wo64 = Wo.astype(np.float64)
    Wpr = np.empty((E, E, MODES), np.float32)
    Wpi = np.empty((E, E, MODES), np.float32)
    for mm in range(MODES):
        ar = Wq64.T @ W1r[:, :, mm].astype(np.float64)
        ai = Wq64.T @ W1i[:, :, mm].astype(np.float64)
        Wpr[:, :, mm] = (ar @ Wo64.T).astype(np.float32)
        Wpi[:, :, mm] = (ai @ Wo64.T).astype(np.float32)

    # Inverse DFT rows g[k = j*8 + t*2 + ri, l]
    cm = np.where(m == 0, 1.0, 2.0)
    g_mat = np.empty((64, L), np.float32)
    for r in range(64):
        c, x, pp = r // 8, (r % 8) // 2, r % 2
        tl, ri = x // 2, x % 2
        mm = 4 * c + 2 * pp + tl
        if ri == 0:
            g_mat[r] = cm[mm] * cos_ml[mm] / L
        else:
            g_mat[r] = -cm[mm] * sin_ml[mm] / L
            if mm == 0:
                g_mat[r] = 0.0  # irfft ignores Im(bin 0)

    out_bias = (
        bo.astype(np.float64)
        + bq.astype(np.float64) @ W1r[:, :, 0].astype(np.float64) @ Wo64.T
    ).astype(np.float32)

    ft_b = ft.astype(BF16)
    g_b = np.vstack([g_mat, g_mat]).astype(BF16)

    in_maps = []
    for c in range(NCORES):
        w_pack = np.empty((ML, 128, 3, NCH, E), np.float32)
        for t in range(ML):
            mm = 4 * c + t
            for ch in range(NCH):
                w_pack[t, :, 0, ch] = Wpr[128 * ch : 128 * (ch + 1), :, mm]
                w_pack[t, :, 1, ch] = Wpi[128 * ch : 128 * (ch + 1), :, mm]
                w_pack[t, :, 2, ch] = -Wpi[128 * ch : 128 * (ch + 1), :, mm]
        w_pack = w_pack.reshape(ML, 128, 3 * NCH * E)
        in_maps.append(
            {
                "q": np.ascontiguousarray(queries[BL * c : BL * (c + 1)]).astype(BF16),
                "ft": ft_b,
                "w": w_pack.astype(BF16),
                "g": g_b,
                "mb": np.broadcast_to(
                    L * out_bias[None, :] if c == 0 else np.zeros((1, E), np.float32),
                    (B, E),
                ).astype(np.float32),
            }
        )
    return in_maps


def kernel(queries, Wq, bq, W1r, W1i, Wo, bo, _trace=False):
    global _nc_cache
    with_bias = bool(np.any(bq) or np.any(bo))
    if with_bias not in _nc_cache:
        _nc_cache[with_bias] = build_nc(with_bias)
    nc = _nc_cache[with_bias]

    in_maps = _host_prep(queries, Wq, bq, W1r, W1i, Wo, bo)
    res = run_bass_kernel_spmd(nc, in_maps, core_ids=list(range(NCORES)), trace=_trace)
    results = res.results
    out = np.concatenate(
        [
            np.asarray(r["out"])
            .reshape(BL, 128, KT, E)
            .transpose(0, 2, 1, 3)
            .reshape(BL, L, E)
            for r in results
        ],
        axis=0,
    )
    if _trace:
        kernel._last = res
    return out.astype(np.float32)
